# revision 7
# baseline (speedup 1.0000x reference)
"""Multi-head attention (B=2, S=2048, D=2048, H=16) on 8 trn2 NeuronCores.

Sharding: core i handles batch i//4 and heads 4*(i%4) .. 4*(i%4)+4.
Each core produces a partial [S, D] out-projection (its 4 heads' contribution);
the host sums the 4 partials per batch and adds b_out (the "all-reduce").

Per-core dataflow (everything transposed so no on-chip transposes are needed):
  input xT = x[b].T  [D, S]
  phase 1: qT[h] = (Wq_h^T @ xT) * 1/sqrt(dk) + bq   [dk=128, S]   (spilled to DRAM)
           kT[h] =  Wk_h^T @ xT + bk                 [dk=128, S]   (spilled)
           v     =  x @ Wv + bv                      [S, 4*dk]     (spilled)
  phase 2: per head, per q-tile of 512 columns:
           sT(kc) = k_chunk @ qT        [128 k, 512 q]   (scoresT, one k-chunk)
           eT(kc) = exp(sT(kc))                          (no max-subtraction:
                                                          scores are O(1))
           sums  += ones^T @ eT(kc)     [128, 512]       (softmax denominator)
           avT   += v_chunk^T^T... i.e. matmul(lhsT=v_chunk, rhs=eT)  -> [dv, q]
           attn_outT = avT * (1/sums)                    (deferred normalization)
  phase 3: out_partial[s, :] += attn_outT_h[:, s_tile]^T @ Wout_h  summed over h

All matmuls run as float32r (full PE throughput at N=512, ~TF32 accuracy),
with fp32 storage and fp32 PSUM accumulation.
"""

import math
import os
import threading

import numpy as np

import concourse.bass as bass
import concourse.tile as tile
from concourse import bacc, mybir
from concourse import bass_utils

F32 = mybir.dt.float32
F32R = mybir.dt.float32r
AF = mybir.ActivationFunctionType

P = 128  # partitions / systolic tile


class Cfg:
    def __init__(self, D=2048, S=2048, HPC=4, S_SLAB=512, QT=512, DOUT_T=512):
        self.D = D          # model dim (contraction for projections)
        self.S = S          # sequence length
        self.HPC = HPC      # heads per core
        self.DK = P         # head dim
        self.S_SLAB = S_SLAB
        self.QT = QT        # q-tile width in phase 2
        self.DOUT_T = DOUT_T
        self.DC = D // P            # contraction chunks
        self.NHC = HPC * self.DK    # per-core qkv head columns
        self.N_SLABS = S // S_SLAB
        self.SC_PER_SLAB = S_SLAB // P
        self.N_QT = S // QT
        self.N_KC = S // P
        self.N_ST = S // P
        self.N_DOUT = D // DOUT_T


def build_bass(cfg: Cfg, num_devices: int = 8):
    """Build the single-core SPMD Bass program."""
    nc = bacc.Bacc(
        "TRN2",
        target_bir_lowering=False,
        debug=False,
        enable_asserts=False,
        num_devices=num_devices,
    )

    xT = nc.dram_tensor("xT", [cfg.D, cfg.S], F32R, kind="ExternalInput").ap()
    wq = nc.dram_tensor("wq", [cfg.D, cfg.NHC], F32R, kind="ExternalInput").ap()
    wk = nc.dram_tensor("wk", [cfg.D, cfg.NHC], F32R, kind="ExternalInput").ap()
    wv = nc.dram_tensor("wv", [cfg.D, cfg.NHC], F32R, kind="ExternalInput").ap()
    wo = nc.dram_tensor("wo", [cfg.NHC, cfg.D], F32R, kind="ExternalInput").ap()
    bq = nc.dram_tensor("bq", [P, cfg.HPC], F32, kind="ExternalInput").ap()
    bk = nc.dram_tensor("bk", [P, cfg.HPC], F32, kind="ExternalInput").ap()
    bvb = nc.dram_tensor("bvb", [P, cfg.NHC], F32, kind="ExternalInput").ap()
    ones = nc.dram_tensor("ones", [P, P], F32R, kind="ExternalInput").ap()
    out = nc.dram_tensor("out", [cfg.S, cfg.D], F32, kind="ExternalOutput").ap()

    xr = xT.rearrange("(c p) s -> p c s", p=P)       # [128, DC, S]
    wqr = wq.rearrange("(c p) n -> p c n", p=P)      # [128, DC, NHC]
    wkr = wk.rearrange("(c p) n -> p c n", p=P)
    wvr = wv.rearrange("(c p) n -> p c n", p=P)
    wor = wo.rearrange("(h p) n -> p h n", p=P)      # [128, HPC, D]

    HGRP = 2                     # k-chunks per score-psum group (2 banks)
    HALF = cfg.N_KC // 2         # k-chunks per exp half-buffer

    with tile.TileContext(nc) as tc:
        with tc.tile_pool(name="dram", bufs=1, space="DRAM") as dram, \
             tc.tile_pool(name="consts", bufs=1) as consts, \
             tc.tile_pool(name="v_pool", bufs=1) as v_pool, \
             tc.tile_pool(name="pre_p", bufs=1) as pre_p:
            qT_d = dram.tile([cfg.HPC, P, cfg.S], F32R, name="qT_d")
            kT_d = dram.tile([cfg.HPC, P, cfg.S], F32R, name="kT_d")

            ones_t = consts.tile([P, P], F32R, name="ones_t")
            nc.sync.dma_start(out=ones_t, in_=ones)
            zb = consts.tile([P, 1], F32, name="zb")
            nc.vector.memset(zb, 0.0)
            bq_t = consts.tile([P, cfg.HPC], F32, name="bq_t")
            nc.sync.dma_start(out=bq_t, in_=bq)
            bk_t = consts.tile([P, cfg.HPC], F32, name="bk_t")
            nc.sync.dma_start(out=bk_t, in_=bk)
            bvb_t = consts.tile([P, cfg.NHC], F32, name="bvb_t")
            nc.sync.dma_start(out=bvb_t, in_=bvb)

            # v stays resident in SBUF across phases 1-2 (no DRAM spill)
            v_sb = v_pool.tile([P, cfg.N_KC, cfg.NHC], F32R, name="v_sb")

            # ---------------- Phase 1: QKV projections ----------------
            with tc.tile_pool(name="w_p", bufs=1) as w_p, \
                 tc.tile_pool(name="wv_p", bufs=3) as wv_p, \
                 tc.tile_pool(name="x_p", bufs=2) as x_p, \
                 tc.tile_pool(name="ev_p", bufs=3) as ev_p, \
                 tc.tile_pool(name="ps1", bufs=2, space="PSUM") as ps1:
                # slab-0 x DMAs go first so the v matmuls can start
                # immediately; the 8MB of Wq/Wk loads stream behind them.
                x0_t = x_p.tile([P, cfg.DC, cfg.S_SLAB], F32R, name="x_t")
                for c in range(cfg.DC):
                    nc.sync.dma_start(out=x0_t[:, c], in_=xr[:, c, 0:cfg.S_SLAB])
                wq_t = w_p.tile([P, cfg.DC, cfg.NHC], F32R, name="wq_t")
                wk_t = w_p.tile([P, cfg.DC, cfg.NHC], F32R, name="wk_t")
                for h in range(cfg.HPC):
                    nc.gpsimd.dma_start(
                        out=wq_t[:, :, h * P:(h + 1) * P],
                        in_=wqr[:, :, h * P:(h + 1) * P])
                    nc.gpsimd.dma_start(
                        out=wk_t[:, :, h * P:(h + 1) * P],
                        in_=wkr[:, :, h * P:(h + 1) * P])

                for slab in range(cfg.N_SLABS):
                    s0 = slab * cfg.S_SLAB
                    s1 = s0 + cfg.S_SLAB
                    if slab == 0:
                        x_t = x0_t
                    else:
                        x_t = x_p.tile([P, cfg.DC, cfg.S_SLAB], F32R, name="x_t")
                        for c in range(cfg.DC):
                            nc.sync.dma_start(out=x_t[:, c], in_=xr[:, c, s0:s1])

                    # v[s, (h dv)] for this slab: Wv chunks streamed, psums
                    # per s-chunk accumulate with c outermost.
                    pvs = [
                        ps1.tile([P, cfg.NHC], F32, name="pv", bufs=cfg.SC_PER_SLAB)
                        for _ in range(cfg.SC_PER_SLAB)
                    ]
                    for c in range(cfg.DC):
                        wv_ct = wv_p.tile([P, cfg.NHC], F32R, name="wv_ct")
                        nc.gpsimd.dma_start(out=wv_ct, in_=wvr[:, c])
                        for sc in range(cfg.SC_PER_SLAB):
                            nc.tensor.matmul(
                                pvs[sc],
                                lhsT=x_t[:, c, sc * P:(sc + 1) * P],
                                rhs=wv_ct,
                                start=(c == 0),
                                stop=(c == cfg.DC - 1),
                            )
                    for sc in range(cfg.SC_PER_SLAB):
                        nc.vector.tensor_add(
                            v_sb[:, slab * cfg.SC_PER_SLAB + sc, :], pvs[sc], bvb_t
                        )

                    # qT / kT per head for this slab
                    for h in range(cfg.HPC):
                        pq = ps1.tile([P, cfg.S_SLAB], F32, name="pq")
                        for c in range(cfg.DC):
                            nc.tensor.matmul(
                                pq,
                                lhsT=wq_t[:, c, h * P:(h + 1) * P],
                                rhs=x_t[:, c],
                                start=(c == 0),
                                stop=(c == cfg.DC - 1),
                            )
                        qt_s = ev_p.tile([P, cfg.S_SLAB], F32R, name="qt_s", tag="qkev")
                        nc.scalar.activation(
                            qt_s, pq, AF.Identity,
                            bias=bq_t[:, h:h + 1],
                            scale=1.0 / math.sqrt(cfg.DK),
                        )
                        nc.sync.dma_start(out=qT_d[h, :, s0:s1], in_=qt_s)

                        pk = ps1.tile([P, cfg.S_SLAB], F32, name="pk")
                        for c in range(cfg.DC):
                            nc.tensor.matmul(
                                pk,
                                lhsT=wk_t[:, c, h * P:(h + 1) * P],
                                rhs=x_t[:, c],
                                start=(c == 0),
                                stop=(c == cfg.DC - 1),
                            )
                        kt_s = ev_p.tile([P, cfg.S_SLAB], F32R, name="kt_s", tag="qkev")
                        nc.scalar.activation(
                            kt_s, pk, AF.Identity,
                            bias=bk_t[:, h:h + 1],
                            scale=1.0,
                        )
                        nc.sync.dma_start(out=kT_d[h, :, s0:s1], in_=kt_s)

                # prefetch head 0's q/k while phase-1 tail still runs
                qh0 = pre_p.tile([P, cfg.S], F32R, name="qh0")
                nc.sync.dma_start(out=qh0, in_=qT_d[0])
                kh0 = pre_p.tile([P, cfg.S], F32R, name="kh0")
                nc.sync.dma_start(out=kh0, in_=kT_d[0])

            # ------------- Phases 2+3 umbrella (attn + Wout live here) ------
            with tc.tile_pool(name="at_p", bufs=1) as at_p, \
                 tc.tile_pool(name="wo_p", bufs=1) as wo_p:
                attn_sb = at_p.tile([P, cfg.HPC, cfg.S], F32R, name="attn_sb")
                wo_t = wo_p.tile([P, cfg.HPC, cfg.D], F32R, name="wo_t")
                nc.gpsimd.dma_start(out=wo_t, in_=wor)

                # ---------------- Phase 2: attention ----------------
                with tc.tile_pool(name="qk_p", bufs=2) as qk_p, \
                     tc.tile_pool(name="exp_p", bufs=2) as exp_p, \
                     tc.tile_pool(name="sm_p", bufs=2) as sm_p, \
                     tc.tile_pool(name="ps_s", bufs=2, space="PSUM") as ps_s, \
                     tc.tile_pool(name="ps_acc", bufs=2, space="PSUM") as ps_acc:
                    def qtile_tail(t):
                        """Finish a q-tile: fold half-1 pair-sum tree, final
                        ones-matmul, reciprocal, normalize into attn_sb.
                        Emitted one q-tile late so the DVE tree overlaps the
                        next q-tile's matmul stream."""
                        ps_av, ps_sum, last_eth, th, tq0, tq1 = t
                        w = HALF
                        while w > 1:
                            w //= 2
                            nc.vector.tensor_add(
                                last_eth[:, 0:w], last_eth[:, 0:w],
                                last_eth[:, w:2 * w]
                            )
                        nc.tensor.matmul(
                            ps_sum, lhsT=ones_t, rhs=last_eth[:, 0, :],
                            start=False, stop=True,
                        )
                        inv = sm_p.tile([P, cfg.QT], F32, name="inv")
                        nc.vector.reciprocal_approx_fast(inv, ps_sum)
                        nc.vector.tensor_mul(
                            attn_sb[:, th, tq0:tq1], ps_av, inv
                        )

                    pending = None
                    for h in range(cfg.HPC):
                        if h == 0:
                            qh_t, kh_t = qh0, kh0
                        else:
                            qh_t = qk_p.tile([P, cfg.S], F32R, name="qh")
                            nc.sync.dma_start(out=qh_t, in_=qT_d[h])
                            kh_t = qk_p.tile([P, cfg.S], F32R, name="kh")
                            nc.sync.dma_start(out=kh_t, in_=kT_d[h])

                        for qt in range(cfg.N_QT):
                            q0 = qt * cfg.QT
                            q1 = q0 + cfg.QT
                            ps_av = ps_acc.tile([P, cfg.QT], F32, name="ps_av")
                            ps_sum = ps_acc.tile([P, cfg.QT], F32, name="ps_sum")
                            # Hybrid softmax denominator: half 0's exp slices
                            # feed ones-matmuls directly (releases the exp
                            # buffer promptly); half 1 is pair-summed on DVE
                            # (overlaps next q-tile) and folded in with one
                            # final ones-matmul.
                            last_eth = None
                            for half in range(2):
                                eth = exp_p.tile([P, HALF, cfg.QT], F32R, name="eth")
                                for g in range(HALF // HGRP):
                                    st2 = ps_s.tile([P, HGRP, cfg.QT], F32, name="st2")
                                    for j in range(HGRP):
                                        kc = half * HALF + g * HGRP + j
                                        nc.tensor.matmul(
                                            st2[:, j],
                                            lhsT=kh_t[:, kc * P:(kc + 1) * P],
                                            rhs=qh_t[:, q0:q1],
                                            start=True,
                                            stop=True,
                                        )
                                    nc.scalar.activation(
                                        eth[:, g * HGRP:(g + 1) * HGRP, :], st2,
                                        AF.Exp, bias=zb, scale=1.0,
                                    )
                                    for j in range(HGRP):
                                        kc = half * HALF + g * HGRP + j
                                        nc.tensor.matmul(
                                            ps_av,
                                            lhsT=v_sb[:, kc, h * P:(h + 1) * P],
                                            rhs=eth[:, g * HGRP + j, :],
                                            start=(kc == 0),
                                            stop=(kc == cfg.N_KC - 1),
                                        )
                                        if half == 0:
                                            nc.tensor.matmul(
                                                ps_sum, lhsT=ones_t,
                                                rhs=eth[:, g * HGRP + j, :],
                                                start=(kc == 0),
                                                stop=False,
                                            )
                                last_eth = eth
                            if pending is not None:
                                qtile_tail(pending)
                            pending = (ps_av, ps_sum, last_eth, h, q0, q1)
                    qtile_tail(pending)

                # ---------------- Phase 3: output projection ----------------
                with tc.tile_pool(name="o_p", bufs=4) as o_p, \
                     tc.tile_pool(name="ps3", bufs=2 * cfg.N_DOUT, space="PSUM") as ps3:
                    for st_i in range(cfg.N_ST):
                        pos = [
                            ps3.tile([P, cfg.DOUT_T], F32, name="po")
                            for _ in range(cfg.N_DOUT)
                        ]
                        for h in range(cfg.HPC):
                            for dt in range(cfg.N_DOUT):
                                nc.tensor.matmul(
                                    pos[dt],
                                    lhsT=attn_sb[:, h, st_i * P:(st_i + 1) * P],
                                    rhs=wo_t[:, h, dt * cfg.DOUT_T:(dt + 1) * cfg.DOUT_T],
                                    start=(h == 0),
                                    stop=(h == cfg.HPC - 1),
                                )
                        for dt in range(cfg.N_DOUT):
                            ot = o_p.tile([P, cfg.DOUT_T], F32, name="ot")
                            if dt % 2 == 0:
                                nc.scalar.copy(ot, pos[dt])
                            else:
                                nc.vector.tensor_copy(ot, pos[dt])
                            nc.sync.dma_start(
                                out=out[st_i * P:(st_i + 1) * P,
                                        dt * cfg.DOUT_T:(dt + 1) * cfg.DOUT_T],
                                in_=ot,
                            )

    nc.compile()
    return nc


def make_in_maps(x, W_qkv, b_qkv, cfg: Cfg, W_out):
    """Shard the full inputs into 8 per-core input dicts.

    Reference layout: qkv.reshape(B, S, H, 3*dk) -> head h owns W_qkv columns
    [h*3*dk, (h+1)*3*dk), split q | k | v within the group of 3*dk.
    """
    DK = cfg.DK
    NHC = cfg.NHC
    in_maps = []
    n_heads_total = W_qkv.shape[1] // (3 * DK)
    n_groups = n_heads_total // cfg.HPC
    for core in range(8):
        b = core // n_groups
        g = core % n_groups
        heads = list(range(g * cfg.HPC, (g + 1) * cfg.HPC))
        xTc = np.ascontiguousarray(x[b].T)
        wq_c = np.concatenate(
            [W_qkv[:, gh * 3 * DK:gh * 3 * DK + DK] for gh in heads], axis=1)
        wk_c = np.concatenate(
            [W_qkv[:, gh * 3 * DK + DK:gh * 3 * DK + 2 * DK] for gh in heads], axis=1)
        wv_c = np.concatenate(
            [W_qkv[:, gh * 3 * DK + 2 * DK:gh * 3 * DK + 3 * DK] for gh in heads], axis=1)
        wo_c = np.ascontiguousarray(W_out[g * NHC:(g + 1) * NHC, :])
        bq_c = np.stack(
            [b_qkv[gh * 3 * DK:gh * 3 * DK + DK] for gh in heads], axis=1
        ) / math.sqrt(DK)
        bk_c = np.stack(
            [b_qkv[gh * 3 * DK + DK:gh * 3 * DK + 2 * DK] for gh in heads], axis=1)
        bv_flat = np.concatenate(
            [b_qkv[gh * 3 * DK + 2 * DK:gh * 3 * DK + 3 * DK] for gh in heads])
        bvb_c = np.broadcast_to(bv_flat[None, :], (P, NHC))
        in_maps.append({
            "xT": xTc.astype(np.float32),
            "wq": np.ascontiguousarray(wq_c).astype(np.float32),
            "wk": np.ascontiguousarray(wk_c).astype(np.float32),
            "wv": np.ascontiguousarray(wv_c).astype(np.float32),
            "wo": wo_c.astype(np.float32),
            "bq": np.ascontiguousarray(bq_c).astype(np.float32),
            "bk": np.ascontiguousarray(bk_c).astype(np.float32),
            "bvb": np.ascontiguousarray(bvb_c).astype(np.float32),
            "ones": np.ones((P, P), dtype=np.float32),
        })
    return in_maps


_build_lock = threading.Lock()
_cached_nc = None
LAST_RESULTS = None  # BassKernelResults of the most recent kernel() call


def _get_nc():
    global _cached_nc
    with _build_lock:
        if _cached_nc is None:
            _cached_nc = build_bass(Cfg(), num_devices=8)
    return _cached_nc


def kernel(x, W_qkv, b_qkv, W_out, b_out):
    global LAST_RESULTS
    x = np.asarray(x, dtype=np.float32)
    W_qkv = np.asarray(W_qkv, dtype=np.float32)
    b_qkv = np.asarray(b_qkv, dtype=np.float32)
    W_out = np.asarray(W_out, dtype=np.float32)
    b_out = np.asarray(b_out, dtype=np.float32)

    cfg = Cfg()
    nc = _get_nc()
    in_maps = make_in_maps(x, W_qkv, b_qkv, cfg, W_out)
    trace = bool(int(os.environ.get("KERNEL_TRACE", "0")))
    res = bass_utils.run_bass_kernel_spmd(
        nc, in_maps, core_ids=list(range(8)), trace=trace,
        stitch_traces=False,
    )
    LAST_RESULTS = res
    B = x.shape[0]
    out = np.empty((B, cfg.S, cfg.D), dtype=np.float32)
    n_groups = 8 // B
    for b in range(B):
        acc = res.results[b * n_groups]["out"].copy()
        for g in range(1, n_groups):
            acc += res.results[b * n_groups + g]["out"]
        out[b] = acc + b_out[None, :]
    return out


# revision 9
# speedup vs baseline: 1.1425x; 1.1425x over previous
"""Multi-head attention (B=2, S=2048, D=2048, H=16) on 8 trn2 NeuronCores.

Sharding: core i handles batch i//4 and heads 4*(i%4) .. 4*(i%4)+4.
Each core produces a partial [S, D] out-projection (its 4 heads' contribution);
the host sums the 4 partials per batch and adds b_out (the "all-reduce").

Per-core dataflow (everything transposed so no on-chip transposes are needed):
  input xT = x[b].T  [D, S]
  phase 1: qT[h] = (Wq_h^T @ xT) * 1/sqrt(dk) + bq   [dk=128, S]   (spilled to DRAM)
           kT[h] =  Wk_h^T @ xT + bk                 [dk=128, S]   (spilled)
           v     =  x @ Wv + bv                      [S, 4*dk]     (spilled)
  phase 2: per head, per q-tile of 512 columns:
           sT(kc) = k_chunk @ qT        [128 k, 512 q]   (scoresT, one k-chunk)
           eT(kc) = exp(sT(kc))                          (no max-subtraction:
                                                          scores are O(1))
           sums  += ones^T @ eT(kc)     [128, 512]       (softmax denominator)
           avT   += v_chunk^T^T... i.e. matmul(lhsT=v_chunk, rhs=eT)  -> [dv, q]
           attn_outT = avT * (1/sums)                    (deferred normalization)
  phase 3: out_partial[s, :] += attn_outT_h[:, s_tile]^T @ Wout_h  summed over h

All matmuls run as float32r (full PE throughput at N=512, ~TF32 accuracy),
with fp32 storage and fp32 PSUM accumulation.
"""

import math
import os
import threading

import numpy as np

import concourse.bass as bass
import concourse.tile as tile
from concourse import bacc, mybir
from concourse import bass_utils

F32 = mybir.dt.float32
F32R = mybir.dt.float32r
AF = mybir.ActivationFunctionType

P = 128  # partitions / systolic tile


class Cfg:
    def __init__(self, D=2048, S=2048, HPC=4, S_SLAB=512, QT=512, DOUT_T=512):
        self.D = D          # model dim (contraction for projections)
        self.S = S          # sequence length
        self.HPC = HPC      # heads per core
        self.DK = P         # head dim
        self.S_SLAB = S_SLAB
        self.QT = QT        # q-tile width in phase 2
        self.DOUT_T = DOUT_T
        self.DC = D // P            # contraction chunks
        self.NHC = HPC * self.DK    # per-core qkv head columns
        self.N_SLABS = S // S_SLAB
        self.SC_PER_SLAB = S_SLAB // P
        self.N_QT = S // QT
        self.N_KC = S // P
        self.N_ST = S // P
        self.N_DOUT = D // DOUT_T


def build_bass(cfg: Cfg, num_devices: int = 8):
    """Build the single-core SPMD Bass program."""
    nc = bacc.Bacc(
        "TRN2",
        target_bir_lowering=False,
        debug=False,
        enable_asserts=False,
        num_devices=num_devices,
    )

    xT = nc.dram_tensor("xT", [cfg.D, cfg.S], F32R, kind="ExternalInput").ap()
    wq = nc.dram_tensor("wq", [cfg.D, cfg.NHC], F32R, kind="ExternalInput").ap()
    wk = nc.dram_tensor("wk", [cfg.D, cfg.NHC], F32R, kind="ExternalInput").ap()
    wv = nc.dram_tensor("wv", [cfg.D, cfg.NHC], F32R, kind="ExternalInput").ap()
    wo = nc.dram_tensor("wo", [cfg.NHC, cfg.D], F32R, kind="ExternalInput").ap()
    bq = nc.dram_tensor("bq", [P, cfg.HPC], F32, kind="ExternalInput").ap()
    bk = nc.dram_tensor("bk", [P, cfg.HPC], F32, kind="ExternalInput").ap()
    bvb = nc.dram_tensor("bvb", [P, cfg.NHC], F32, kind="ExternalInput").ap()
    ones = nc.dram_tensor("ones", [P, P], F32R, kind="ExternalInput").ap()
    out = nc.dram_tensor("out", [cfg.S, cfg.D], F32, kind="ExternalOutput").ap()

    xr = xT.rearrange("(c p) s -> p c s", p=P)       # [128, DC, S]
    wqr = wq.rearrange("(c p) n -> p c n", p=P)      # [128, DC, NHC]
    wkr = wk.rearrange("(c p) n -> p c n", p=P)
    wvr = wv.rearrange("(c p) n -> p c n", p=P)
    wor = wo.rearrange("(h p) n -> p h n", p=P)      # [128, HPC, D]

    QCH = max(1, cfg.N_KC // 4)  # k-chunks per exp quarter-buffer
    NQTR = cfg.N_KC // QCH       # quarter-buffers per q-tile
    HGRP = min(2, QCH)           # k-chunks per score-psum group (banks)

    with tile.TileContext(nc) as tc:
        with tc.tile_pool(name="dram", bufs=1, space="DRAM") as dram, \
             tc.tile_pool(name="consts", bufs=1) as consts, \
             tc.tile_pool(name="v_pool", bufs=1) as v_pool, \
             tc.tile_pool(name="pre_p", bufs=1) as pre_p:
            qT_d = dram.tile([cfg.HPC, P, cfg.S], F32R, name="qT_d")
            kT_d = dram.tile([cfg.HPC, P, cfg.S], F32R, name="kT_d")

            ones_t = consts.tile([P, P], F32R, name="ones_t")
            nc.sync.dma_start(out=ones_t, in_=ones)
            zb = consts.tile([P, 1], F32, name="zb")
            nc.vector.memset(zb, 0.0)
            bq_t = consts.tile([P, cfg.HPC], F32, name="bq_t")
            nc.sync.dma_start(out=bq_t, in_=bq)
            bk_t = consts.tile([P, cfg.HPC], F32, name="bk_t")
            nc.sync.dma_start(out=bk_t, in_=bk)
            bvb_t = consts.tile([P, cfg.NHC], F32, name="bvb_t")
            nc.sync.dma_start(out=bvb_t, in_=bvb)

            # v stays resident in SBUF across phases 1-2 (no DRAM spill)
            v_sb = v_pool.tile([P, cfg.N_KC, cfg.NHC], F32R, name="v_sb")

            # ---------------- Phase 1: QKV projections ----------------
            with tc.tile_pool(name="w_p", bufs=1) as w_p, \
                 tc.tile_pool(name="wv_p", bufs=3) as wv_p, \
                 tc.tile_pool(name="x_p", bufs=2) as x_p, \
                 tc.tile_pool(name="ev_p", bufs=3) as ev_p, \
                 tc.tile_pool(name="ps1", bufs=2, space="PSUM") as ps1:
                # slab-0 x DMAs go first so the v matmuls can start
                # immediately; the 8MB of Wq/Wk loads stream behind them.
                x0_t = x_p.tile([P, cfg.DC, cfg.S_SLAB], F32R, name="x_t")
                for c in range(cfg.DC):
                    nc.sync.dma_start(out=x0_t[:, c], in_=xr[:, c, 0:cfg.S_SLAB])
                wq_t = w_p.tile([P, cfg.DC, cfg.NHC], F32R, name="wq_t")
                wk_t = w_p.tile([P, cfg.DC, cfg.NHC], F32R, name="wk_t")
                for h in range(cfg.HPC):
                    nc.gpsimd.dma_start(
                        out=wq_t[:, :, h * P:(h + 1) * P],
                        in_=wqr[:, :, h * P:(h + 1) * P])
                    nc.gpsimd.dma_start(
                        out=wk_t[:, :, h * P:(h + 1) * P],
                        in_=wkr[:, :, h * P:(h + 1) * P])

                for slab in range(cfg.N_SLABS):
                    s0 = slab * cfg.S_SLAB
                    s1 = s0 + cfg.S_SLAB
                    if slab == 0:
                        x_t = x0_t
                    else:
                        x_t = x_p.tile([P, cfg.DC, cfg.S_SLAB], F32R, name="x_t")
                        for c in range(cfg.DC):
                            nc.sync.dma_start(out=x_t[:, c], in_=xr[:, c, s0:s1])

                    # v[s, (h dv)] for this slab: Wv chunks streamed, psums
                    # per s-chunk accumulate with c outermost.
                    pvs = [
                        ps1.tile([P, cfg.NHC], F32, name="pv", bufs=cfg.SC_PER_SLAB)
                        for _ in range(cfg.SC_PER_SLAB)
                    ]
                    for c in range(cfg.DC):
                        wv_ct = wv_p.tile([P, cfg.NHC], F32R, name="wv_ct")
                        nc.scalar.dma_start(out=wv_ct, in_=wvr[:, c])
                        for sc in range(cfg.SC_PER_SLAB):
                            nc.tensor.matmul(
                                pvs[sc],
                                lhsT=x_t[:, c, sc * P:(sc + 1) * P],
                                rhs=wv_ct,
                                start=(c == 0),
                                stop=(c == cfg.DC - 1),
                            )
                    for sc in range(cfg.SC_PER_SLAB):
                        nc.vector.tensor_add(
                            v_sb[:, slab * cfg.SC_PER_SLAB + sc, :], pvs[sc], bvb_t
                        )

                    # qT / kT per head for this slab
                    for h in range(cfg.HPC):
                        pq = ps1.tile([P, cfg.S_SLAB], F32, name="pq")
                        for c in range(cfg.DC):
                            nc.tensor.matmul(
                                pq,
                                lhsT=wq_t[:, c, h * P:(h + 1) * P],
                                rhs=x_t[:, c],
                                start=(c == 0),
                                stop=(c == cfg.DC - 1),
                            )
                        qt_s = ev_p.tile([P, cfg.S_SLAB], F32R, name="qt_s", tag="qkev")
                        nc.scalar.activation(
                            qt_s, pq, AF.Identity,
                            bias=bq_t[:, h:h + 1],
                            scale=1.0 / math.sqrt(cfg.DK),
                        )
                        nc.sync.dma_start(out=qT_d[h, :, s0:s1], in_=qt_s)

                        pk = ps1.tile([P, cfg.S_SLAB], F32, name="pk")
                        for c in range(cfg.DC):
                            nc.tensor.matmul(
                                pk,
                                lhsT=wk_t[:, c, h * P:(h + 1) * P],
                                rhs=x_t[:, c],
                                start=(c == 0),
                                stop=(c == cfg.DC - 1),
                            )
                        kt_s = ev_p.tile([P, cfg.S_SLAB], F32R, name="kt_s", tag="qkev")
                        nc.scalar.activation(
                            kt_s, pk, AF.Identity,
                            bias=bk_t[:, h:h + 1],
                            scale=1.0,
                        )
                        nc.sync.dma_start(out=kT_d[h, :, s0:s1], in_=kt_s)

                # prefetch head 0's q/k while phase-1 tail still runs
                qh0 = pre_p.tile([P, cfg.S], F32R, name="qh0")
                nc.sync.dma_start(out=qh0, in_=qT_d[0])
                kh0 = pre_p.tile([P, cfg.S], F32R, name="kh0")
                nc.sync.dma_start(out=kh0, in_=kT_d[0])

            # ------------- Phases 2+3 umbrella (attn + Wout live here) ------
            with tc.tile_pool(name="at_p", bufs=1) as at_p, \
                 tc.tile_pool(name="wo_p", bufs=1) as wo_p:
                attn_sb = at_p.tile([P, cfg.HPC, cfg.S], F32R, name="attn_sb")
                wo_t = wo_p.tile([P, cfg.HPC, cfg.D], F32R, name="wo_t")
                nc.gpsimd.dma_start(out=wo_t, in_=wor)

                # ---------------- Phase 2: attention ----------------
                with tc.tile_pool(name="qk_p", bufs=2) as qk_p, \
                     tc.tile_pool(name="exp_p", bufs=4) as exp_p, \
                     tc.tile_pool(name="sm_p", bufs=2) as sm_p, \
                     tc.tile_pool(name="ps_s", bufs=2, space="PSUM") as ps_s, \
                     tc.tile_pool(name="ps_acc", bufs=2, space="PSUM") as ps_acc:
                    def qtile_tail(t):
                        """Finish a q-tile one iteration late: final
                        ones-matmul on the last quarter-sum, reciprocal,
                        normalize into attn_sb."""
                        ps_av, ps_sum, last_qsum, th, tq0, tq1 = t
                        nc.tensor.matmul(
                            ps_sum, lhsT=ones_t, rhs=last_qsum,
                            start=(NQTR == 1), stop=True,
                        )
                        inv = sm_p.tile([P, cfg.QT], F32, name="inv")
                        nc.vector.reciprocal_approx_fast(inv, ps_sum)
                        nc.vector.tensor_mul(
                            attn_sb[:, th, tq0:tq1], ps_av, inv
                        )

                    pending = None
                    for h in range(cfg.HPC):
                        if h == 0:
                            qh_t, kh_t = qh0, kh0
                        else:
                            qh_t = qk_p.tile([P, cfg.S], F32R, name="qh")
                            nc.sync.dma_start(out=qh_t, in_=qT_d[h])
                            kh_t = qk_p.tile([P, cfg.S], F32R, name="kh")
                            nc.sync.dma_start(out=kh_t, in_=kT_d[h])

                        for qt in range(cfg.N_QT):
                            q0 = qt * cfg.QT
                            q1 = q0 + cfg.QT
                            ps_av = ps_acc.tile([P, cfg.QT], F32, name="ps_av")
                            ps_sum = ps_acc.tile([P, cfg.QT], F32, name="ps_sum")
                            # Per quarter: scores -> exp -> AV matmuls, then an
                            # in-place DVE pair-sum tree; the quarter-sum feeds
                            # a ones-matmul one quarter later (lag hides the
                            # tree latency from the PE stream).
                            qsums = []
                            for qtr in range(NQTR):
                                eth = exp_p.tile([P, QCH, cfg.QT], F32R, name="eth")
                                for g in range(QCH // HGRP):
                                    st2 = ps_s.tile([P, HGRP, cfg.QT], F32, name="st2")
                                    for j in range(HGRP):
                                        kc = qtr * QCH + g * HGRP + j
                                        nc.tensor.matmul(
                                            st2[:, j],
                                            lhsT=kh_t[:, kc * P:(kc + 1) * P],
                                            rhs=qh_t[:, q0:q1],
                                            start=True,
                                            stop=True,
                                        )
                                    nc.scalar.activation(
                                        eth[:, g * HGRP:(g + 1) * HGRP, :], st2,
                                        AF.Exp, bias=zb, scale=1.0,
                                    )
                                    for j in range(HGRP):
                                        kc = qtr * QCH + g * HGRP + j
                                        nc.tensor.matmul(
                                            ps_av,
                                            lhsT=v_sb[:, kc, h * P:(h + 1) * P],
                                            rhs=eth[:, g * HGRP + j, :],
                                            start=(kc == 0),
                                            stop=(kc == cfg.N_KC - 1),
                                        )
                                w = QCH
                                while w > 1:
                                    w //= 2
                                    nc.vector.tensor_add(
                                        eth[:, 0:w], eth[:, 0:w], eth[:, w:2 * w]
                                    )
                                qsums.append(eth[:, 0, :])
                                if qtr >= 1:
                                    nc.tensor.matmul(
                                        ps_sum, lhsT=ones_t, rhs=qsums[qtr - 1],
                                        start=(qtr - 1 == 0), stop=False,
                                    )
                            if pending is not None:
                                qtile_tail(pending)
                            pending = (ps_av, ps_sum, qsums[NQTR - 1], h, q0, q1)
                    qtile_tail(pending)

                # ---------------- Phase 3: output projection ----------------
                with tc.tile_pool(name="o_p", bufs=4) as o_p, \
                     tc.tile_pool(name="ps3", bufs=2 * cfg.N_DOUT, space="PSUM") as ps3:
                    for st_i in range(cfg.N_ST):
                        pos = [
                            ps3.tile([P, cfg.DOUT_T], F32, name="po")
                            for _ in range(cfg.N_DOUT)
                        ]
                        for h in range(cfg.HPC):
                            for dt in range(cfg.N_DOUT):
                                nc.tensor.matmul(
                                    pos[dt],
                                    lhsT=attn_sb[:, h, st_i * P:(st_i + 1) * P],
                                    rhs=wo_t[:, h, dt * cfg.DOUT_T:(dt + 1) * cfg.DOUT_T],
                                    start=(h == 0),
                                    stop=(h == cfg.HPC - 1),
                                )
                        for dt in range(cfg.N_DOUT):
                            ot = o_p.tile([P, cfg.DOUT_T], F32, name="ot")
                            if dt % 2 == 0:
                                nc.scalar.copy(ot, pos[dt])
                            else:
                                nc.vector.tensor_copy(ot, pos[dt])
                            nc.sync.dma_start(
                                out=out[st_i * P:(st_i + 1) * P,
                                        dt * cfg.DOUT_T:(dt + 1) * cfg.DOUT_T],
                                in_=ot,
                            )

    nc.compile()
    return nc


def make_in_maps(x, W_qkv, b_qkv, cfg: Cfg, W_out):
    """Shard the full inputs into 8 per-core input dicts.

    Reference layout: qkv.reshape(B, S, H, 3*dk) -> head h owns W_qkv columns
    [h*3*dk, (h+1)*3*dk), split q | k | v within the group of 3*dk.
    """
    DK = cfg.DK
    NHC = cfg.NHC
    in_maps = []
    n_heads_total = W_qkv.shape[1] // (3 * DK)
    n_groups = n_heads_total // cfg.HPC
    for core in range(8):
        b = core // n_groups
        g = core % n_groups
        heads = list(range(g * cfg.HPC, (g + 1) * cfg.HPC))
        xTc = np.ascontiguousarray(x[b].T)
        wq_c = np.concatenate(
            [W_qkv[:, gh * 3 * DK:gh * 3 * DK + DK] for gh in heads], axis=1)
        wk_c = np.concatenate(
            [W_qkv[:, gh * 3 * DK + DK:gh * 3 * DK + 2 * DK] for gh in heads], axis=1)
        wv_c = np.concatenate(
            [W_qkv[:, gh * 3 * DK + 2 * DK:gh * 3 * DK + 3 * DK] for gh in heads], axis=1)
        wo_c = np.ascontiguousarray(W_out[g * NHC:(g + 1) * NHC, :])
        bq_c = np.stack(
            [b_qkv[gh * 3 * DK:gh * 3 * DK + DK] for gh in heads], axis=1
        ) / math.sqrt(DK)
        bk_c = np.stack(
            [b_qkv[gh * 3 * DK + DK:gh * 3 * DK + 2 * DK] for gh in heads], axis=1)
        bv_flat = np.concatenate(
            [b_qkv[gh * 3 * DK + 2 * DK:gh * 3 * DK + 3 * DK] for gh in heads])
        bvb_c = np.broadcast_to(bv_flat[None, :], (P, NHC))
        in_maps.append({
            "xT": xTc.astype(np.float32),
            "wq": np.ascontiguousarray(wq_c).astype(np.float32),
            "wk": np.ascontiguousarray(wk_c).astype(np.float32),
            "wv": np.ascontiguousarray(wv_c).astype(np.float32),
            "wo": wo_c.astype(np.float32),
            "bq": np.ascontiguousarray(bq_c).astype(np.float32),
            "bk": np.ascontiguousarray(bk_c).astype(np.float32),
            "bvb": np.ascontiguousarray(bvb_c).astype(np.float32),
            "ones": np.ones((P, P), dtype=np.float32),
        })
    return in_maps


_build_lock = threading.Lock()
_cached_nc = None
LAST_RESULTS = None  # BassKernelResults of the most recent kernel() call


def _get_nc():
    global _cached_nc
    with _build_lock:
        if _cached_nc is None:
            _cached_nc = build_bass(Cfg(), num_devices=8)
    return _cached_nc


def kernel(x, W_qkv, b_qkv, W_out, b_out):
    global LAST_RESULTS
    x = np.asarray(x, dtype=np.float32)
    W_qkv = np.asarray(W_qkv, dtype=np.float32)
    b_qkv = np.asarray(b_qkv, dtype=np.float32)
    W_out = np.asarray(W_out, dtype=np.float32)
    b_out = np.asarray(b_out, dtype=np.float32)

    cfg = Cfg()
    nc = _get_nc()
    in_maps = make_in_maps(x, W_qkv, b_qkv, cfg, W_out)
    trace = bool(int(os.environ.get("KERNEL_TRACE", "0")))
    res = bass_utils.run_bass_kernel_spmd(
        nc, in_maps, core_ids=list(range(8)), trace=trace,
        stitch_traces=False,
    )
    LAST_RESULTS = res
    B = x.shape[0]
    out = np.empty((B, cfg.S, cfg.D), dtype=np.float32)
    n_groups = 8 // B
    for b in range(B):
        acc = res.results[b * n_groups]["out"].copy()
        for g in range(1, n_groups):
            acc += res.results[b * n_groups + g]["out"]
        out[b] = acc + b_out[None, :]
    return out


# revision 11
# speedup vs baseline: 1.1840x; 1.0363x over previous
"""Multi-head attention (B=2, S=2048, D=2048, H=16) on 8 trn2 NeuronCores.

Sharding: core i handles batch i//4 and heads 4*(i%4) .. 4*(i%4)+4.
Each core produces a partial [S, D] out-projection (its 4 heads' contribution);
the host sums the 4 partials per batch and adds b_out (the "all-reduce").

Per-core dataflow (everything transposed so no on-chip transposes are needed):
  input xT = x[b].T  [D, S]
  phase 1: qT[h] = (Wq_h^T @ xT) * 1/sqrt(dk) + bq   [dk=128, S]   (spilled to DRAM)
           kT[h] =  Wk_h^T @ xT + bk                 [dk=128, S]   (spilled)
           v     =  x @ Wv + bv                      [S, 4*dk]     (spilled)
  phase 2: per head, per q-tile of 512 columns:
           sT(kc) = k_chunk @ qT        [128 k, 512 q]   (scoresT, one k-chunk)
           eT(kc) = exp(sT(kc))                          (no max-subtraction:
                                                          scores are O(1))
           sums  += ones^T @ eT(kc)     [128, 512]       (softmax denominator)
           avT   += v_chunk^T^T... i.e. matmul(lhsT=v_chunk, rhs=eT)  -> [dv, q]
           attn_outT = avT * (1/sums)                    (deferred normalization)
  phase 3: out_partial[s, :] += attn_outT_h[:, s_tile]^T @ Wout_h  summed over h

All matmuls run as float32r (full PE throughput at N=512, ~TF32 accuracy),
with fp32 storage and fp32 PSUM accumulation.
"""

import math
import os
import threading

import numpy as np

import concourse.bass as bass
import concourse.tile as tile
from concourse import bacc, mybir
from concourse import bass_utils

F32 = mybir.dt.float32
F32R = mybir.dt.float32r
AF = mybir.ActivationFunctionType

P = 128  # partitions / systolic tile


class Cfg:
    def __init__(self, D=2048, S=2048, HPC=4, S_SLAB=512, QT=512, DOUT_T=512):
        self.D = D          # model dim (contraction for projections)
        self.S = S          # sequence length
        self.HPC = HPC      # heads per core
        self.DK = P         # head dim
        self.S_SLAB = S_SLAB
        self.QT = QT        # q-tile width in phase 2
        self.DOUT_T = DOUT_T
        self.DC = D // P            # contraction chunks
        self.NHC = HPC * self.DK    # per-core qkv head columns
        self.N_SLABS = S // S_SLAB
        self.SC_PER_SLAB = S_SLAB // P
        self.N_QT = S // QT
        self.N_KC = S // P
        self.N_ST = S // P
        self.N_DOUT = D // DOUT_T


def build_bass(cfg: Cfg, num_devices: int = 8):
    """Build the single-core SPMD Bass program."""
    nc = bacc.Bacc(
        "TRN2",
        target_bir_lowering=False,
        debug=False,
        enable_asserts=False,
        num_devices=num_devices,
    )

    xT = nc.dram_tensor("xT", [cfg.D, cfg.S], F32R, kind="ExternalInput").ap()
    wq = nc.dram_tensor("wq", [cfg.D, cfg.NHC], F32R, kind="ExternalInput").ap()
    wk = nc.dram_tensor("wk", [cfg.D, cfg.NHC], F32R, kind="ExternalInput").ap()
    wv = nc.dram_tensor("wv", [cfg.D, cfg.NHC], F32R, kind="ExternalInput").ap()
    wo = nc.dram_tensor("wo", [cfg.NHC, cfg.D], F32R, kind="ExternalInput").ap()
    bq = nc.dram_tensor("bq", [P, cfg.HPC], F32, kind="ExternalInput").ap()
    bk = nc.dram_tensor("bk", [P, cfg.HPC], F32, kind="ExternalInput").ap()
    bvb = nc.dram_tensor("bvb", [P, cfg.NHC], F32, kind="ExternalInput").ap()
    ones = nc.dram_tensor("ones", [P, P], F32R, kind="ExternalInput").ap()
    out = nc.dram_tensor("out", [cfg.S, cfg.D], F32, kind="ExternalOutput").ap()

    xr = xT.rearrange("(c p) s -> p c s", p=P)       # [128, DC, S]
    wqr = wq.rearrange("(c p) n -> p c n", p=P)      # [128, DC, NHC]
    wkr = wk.rearrange("(c p) n -> p c n", p=P)
    wvr = wv.rearrange("(c p) n -> p c n", p=P)
    wor = wo.rearrange("(h p) n -> p h n", p=P)      # [128, HPC, D]

    QCH = max(1, cfg.N_KC // 4)  # k-chunks per exp quarter-buffer
    NQTR = cfg.N_KC // QCH       # quarter-buffers per q-tile
    HGRP = min(2, QCH)           # k-chunks per score-psum group (banks)

    with tile.TileContext(nc) as tc:
        with tc.tile_pool(name="dram", bufs=1, space="DRAM") as dram, \
             tc.tile_pool(name="consts", bufs=1) as consts, \
             tc.tile_pool(name="v_pool", bufs=1) as v_pool, \
             tc.tile_pool(name="pre_p", bufs=1) as pre_p:
            qT_d = dram.tile([cfg.HPC, P, cfg.S], F32R, name="qT_d")
            kT_d = dram.tile([cfg.HPC, P, cfg.S], F32R, name="kT_d")

            ones_t = consts.tile([P, P], F32R, name="ones_t")
            nc.sync.dma_start(out=ones_t, in_=ones)
            zb = consts.tile([P, 1], F32, name="zb")
            nc.vector.memset(zb, 0.0)
            bq_t = consts.tile([P, cfg.HPC], F32, name="bq_t")
            nc.sync.dma_start(out=bq_t, in_=bq)
            bk_t = consts.tile([P, cfg.HPC], F32, name="bk_t")
            nc.sync.dma_start(out=bk_t, in_=bk)
            bvb_t = consts.tile([P, cfg.NHC], F32, name="bvb_t")
            nc.sync.dma_start(out=bvb_t, in_=bvb)

            # v stays resident in SBUF across phases 1-2 (no DRAM spill)
            v_sb = v_pool.tile([P, cfg.N_KC, cfg.NHC], F32R, name="v_sb")

            # ---------------- Phase 1: QKV projections ----------------
            # Pass A: v = x @ Wv + bv, written straight into resident v_sb.
            # Wv is fully resident; x streams by slab on two DMA queues.
            # Wq/Wk for pass B load in the background on the gpsimd queue.
            with tc.tile_pool(name="w_p", bufs=1) as w_p:
                wq_t = w_p.tile([P, cfg.DC, cfg.NHC], F32R, name="wq_t")
                wk_t = w_p.tile([P, cfg.DC, cfg.NHC], F32R, name="wk_t")

                SLAB_A = max(P, cfg.S_SLAB // 2)
                N_SLABS_A = cfg.S // SLAB_A
                SC_A = SLAB_A // P
                with tc.tile_pool(name="wvf_p", bufs=1) as wvf_p, \
                     tc.tile_pool(name="xa_p", bufs=3) as xa_p, \
                     tc.tile_pool(name="psA", bufs=8, space="PSUM") as psA:
                    wvf_t = wvf_p.tile([P, cfg.DC, cfg.NHC], F32R, name="wvf_t")
                    for c in range(cfg.DC):
                        eng = nc.gpsimd if c % 2 == 0 else nc.scalar
                        eng.dma_start(out=wvf_t[:, c], in_=wvr[:, c])
                    for slab in range(N_SLABS_A):
                        s0 = slab * SLAB_A
                        s1 = s0 + SLAB_A
                        x_t = xa_p.tile([P, cfg.DC, SLAB_A], F32R, name="x_t")
                        for c in range(cfg.DC):
                            eng = nc.sync if c % 2 == 0 else nc.scalar
                            eng.dma_start(out=x_t[:, c], in_=xr[:, c, s0:s1])
                        if slab == 0:
                            # Wq/Wk background loads, behind Wv on gpsimd
                            for h in range(cfg.HPC):
                                nc.gpsimd.dma_start(
                                    out=wq_t[:, :, h * P:(h + 1) * P],
                                    in_=wqr[:, :, h * P:(h + 1) * P])
                                nc.gpsimd.dma_start(
                                    out=wk_t[:, :, h * P:(h + 1) * P],
                                    in_=wkr[:, :, h * P:(h + 1) * P])
                        for sc in range(SC_A):
                            pv = psA.tile([P, cfg.NHC], F32, name="pv")
                            for c in range(cfg.DC):
                                nc.tensor.matmul(
                                    pv,
                                    lhsT=x_t[:, c, sc * P:(sc + 1) * P],
                                    rhs=wvf_t[:, c],
                                    start=(c == 0),
                                    stop=(c == cfg.DC - 1),
                                )
                            nc.vector.tensor_add(
                                v_sb[:, slab * SC_A + sc, :], pv, bvb_t
                            )

                # Pass B: qT/kT projections, spilled to DRAM scratch.
                with tc.tile_pool(name="xb_p", bufs=2) as xb_p, \
                     tc.tile_pool(name="ev_p", bufs=3) as ev_p, \
                     tc.tile_pool(name="psB", bufs=3, space="PSUM") as psB:
                    for slab in range(cfg.N_SLABS):
                        s0 = slab * cfg.S_SLAB
                        s1 = s0 + cfg.S_SLAB
                        x_t = xb_p.tile([P, cfg.DC, cfg.S_SLAB], F32R, name="x_t")
                        for c in range(cfg.DC):
                            eng = nc.sync if c % 2 == 0 else nc.scalar
                            eng.dma_start(out=x_t[:, c], in_=xr[:, c, s0:s1])
                        for h in range(cfg.HPC):
                            pq = psB.tile([P, cfg.S_SLAB], F32, name="pq")
                            for c in range(cfg.DC):
                                nc.tensor.matmul(
                                    pq,
                                    lhsT=wq_t[:, c, h * P:(h + 1) * P],
                                    rhs=x_t[:, c],
                                    start=(c == 0),
                                    stop=(c == cfg.DC - 1),
                                )
                            qt_s = ev_p.tile([P, cfg.S_SLAB], F32R, name="qt_s", tag="qkev")
                            nc.scalar.activation(
                                qt_s, pq, AF.Identity,
                                bias=bq_t[:, h:h + 1],
                                scale=1.0 / math.sqrt(cfg.DK),
                            )
                            nc.sync.dma_start(out=qT_d[h, :, s0:s1], in_=qt_s)

                            pk = psB.tile([P, cfg.S_SLAB], F32, name="pk")
                            for c in range(cfg.DC):
                                nc.tensor.matmul(
                                    pk,
                                    lhsT=wk_t[:, c, h * P:(h + 1) * P],
                                    rhs=x_t[:, c],
                                    start=(c == 0),
                                    stop=(c == cfg.DC - 1),
                                )
                            kt_s = ev_p.tile([P, cfg.S_SLAB], F32R, name="kt_s", tag="qkev")
                            nc.scalar.activation(
                                kt_s, pk, AF.Identity,
                                bias=bk_t[:, h:h + 1],
                                scale=1.0,
                            )
                            nc.sync.dma_start(out=kT_d[h, :, s0:s1], in_=kt_s)

                    # prefetch head 0's q/k while phase-1 tail still runs
                    qh0 = pre_p.tile([P, cfg.S], F32R, name="qh0")
                    nc.sync.dma_start(out=qh0, in_=qT_d[0])
                    kh0 = pre_p.tile([P, cfg.S], F32R, name="kh0")
                    nc.sync.dma_start(out=kh0, in_=kT_d[0])

            # ------------- Phases 2+3 umbrella (attn + Wout live here) ------
            with tc.tile_pool(name="at_p", bufs=1) as at_p, \
                 tc.tile_pool(name="wo_p", bufs=1) as wo_p:
                attn_sb = at_p.tile([P, cfg.HPC, cfg.S], F32R, name="attn_sb")
                wo_t = wo_p.tile([P, cfg.HPC, cfg.D], F32R, name="wo_t")
                nc.gpsimd.dma_start(out=wo_t, in_=wor)

                # ---------------- Phase 2: attention ----------------
                with tc.tile_pool(name="qk_p", bufs=2) as qk_p, \
                     tc.tile_pool(name="exp_p", bufs=4) as exp_p, \
                     tc.tile_pool(name="sm_p", bufs=2) as sm_p, \
                     tc.tile_pool(name="ps_s", bufs=2, space="PSUM") as ps_s, \
                     tc.tile_pool(name="ps_acc", bufs=2, space="PSUM") as ps_acc:
                    def qtile_tail(t):
                        """Finish a q-tile one iteration late: final
                        ones-matmul on the last quarter-sum, reciprocal,
                        normalize into attn_sb."""
                        ps_av, ps_sum, last_qsum, th, tq0, tq1 = t
                        nc.tensor.matmul(
                            ps_sum, lhsT=ones_t, rhs=last_qsum,
                            start=(NQTR == 1), stop=True,
                        )
                        inv = sm_p.tile([P, cfg.QT], F32, name="inv")
                        nc.vector.reciprocal_approx_fast(inv, ps_sum)
                        nc.vector.tensor_mul(
                            attn_sb[:, th, tq0:tq1], ps_av, inv
                        )

                    pending = None
                    for h in range(cfg.HPC):
                        if h == 0:
                            qh_t, kh_t = qh0, kh0
                        else:
                            qh_t = qk_p.tile([P, cfg.S], F32R, name="qh")
                            nc.sync.dma_start(out=qh_t, in_=qT_d[h])
                            kh_t = qk_p.tile([P, cfg.S], F32R, name="kh")
                            nc.sync.dma_start(out=kh_t, in_=kT_d[h])

                        for qt in range(cfg.N_QT):
                            q0 = qt * cfg.QT
                            q1 = q0 + cfg.QT
                            ps_av = ps_acc.tile([P, cfg.QT], F32, name="ps_av")
                            ps_sum = ps_acc.tile([P, cfg.QT], F32, name="ps_sum")
                            # Per quarter: scores -> exp -> AV matmuls, then an
                            # in-place DVE pair-sum tree; the quarter-sum feeds
                            # a ones-matmul one quarter later (lag hides the
                            # tree latency from the PE stream).
                            qsums = []
                            for qtr in range(NQTR):
                                eth = exp_p.tile([P, QCH, cfg.QT], F32R, name="eth")
                                for g in range(QCH // HGRP):
                                    st2 = ps_s.tile([P, HGRP, cfg.QT], F32, name="st2")
                                    for j in range(HGRP):
                                        kc = qtr * QCH + g * HGRP + j
                                        nc.tensor.matmul(
                                            st2[:, j],
                                            lhsT=kh_t[:, kc * P:(kc + 1) * P],
                                            rhs=qh_t[:, q0:q1],
                                            start=True,
                                            stop=True,
                                        )
                                    nc.scalar.activation(
                                        eth[:, g * HGRP:(g + 1) * HGRP, :], st2,
                                        AF.Exp, bias=zb, scale=1.0,
                                    )
                                    for j in range(HGRP):
                                        kc = qtr * QCH + g * HGRP + j
                                        nc.tensor.matmul(
                                            ps_av,
                                            lhsT=v_sb[:, kc, h * P:(h + 1) * P],
                                            rhs=eth[:, g * HGRP + j, :],
                                            start=(kc == 0),
                                            stop=(kc == cfg.N_KC - 1),
                                        )
                                w = QCH
                                while w > 1:
                                    w //= 2
                                    nc.vector.tensor_add(
                                        eth[:, 0:w], eth[:, 0:w], eth[:, w:2 * w]
                                    )
                                qsums.append(eth[:, 0, :])
                                if qtr >= 1:
                                    nc.tensor.matmul(
                                        ps_sum, lhsT=ones_t, rhs=qsums[qtr - 1],
                                        start=(qtr - 1 == 0), stop=False,
                                    )
                            if pending is not None:
                                qtile_tail(pending)
                            pending = (ps_av, ps_sum, qsums[NQTR - 1], h, q0, q1)
                    qtile_tail(pending)

                # ---------------- Phase 3: output projection ----------------
                with tc.tile_pool(name="o_p", bufs=4) as o_p, \
                     tc.tile_pool(name="ps3", bufs=2 * cfg.N_DOUT, space="PSUM") as ps3:
                    for st_i in range(cfg.N_ST):
                        pos = [
                            ps3.tile([P, cfg.DOUT_T], F32, name="po")
                            for _ in range(cfg.N_DOUT)
                        ]
                        for h in range(cfg.HPC):
                            for dt in range(cfg.N_DOUT):
                                nc.tensor.matmul(
                                    pos[dt],
                                    lhsT=attn_sb[:, h, st_i * P:(st_i + 1) * P],
                                    rhs=wo_t[:, h, dt * cfg.DOUT_T:(dt + 1) * cfg.DOUT_T],
                                    start=(h == 0),
                                    stop=(h == cfg.HPC - 1),
                                )
                        for dt in range(cfg.N_DOUT):
                            ot = o_p.tile([P, cfg.DOUT_T], F32, name="ot")
                            if dt % 2 == 0:
                                nc.scalar.copy(ot, pos[dt])
                            else:
                                nc.vector.tensor_copy(ot, pos[dt])
                            nc.sync.dma_start(
                                out=out[st_i * P:(st_i + 1) * P,
                                        dt * cfg.DOUT_T:(dt + 1) * cfg.DOUT_T],
                                in_=ot,
                            )

    nc.compile()
    return nc


def make_in_maps(x, W_qkv, b_qkv, cfg: Cfg, W_out):
    """Shard the full inputs into 8 per-core input dicts.

    Reference layout: qkv.reshape(B, S, H, 3*dk) -> head h owns W_qkv columns
    [h*3*dk, (h+1)*3*dk), split q | k | v within the group of 3*dk.
    """
    DK = cfg.DK
    NHC = cfg.NHC
    in_maps = []
    n_heads_total = W_qkv.shape[1] // (3 * DK)
    n_groups = n_heads_total // cfg.HPC
    for core in range(8):
        b = core // n_groups
        g = core % n_groups
        heads = list(range(g * cfg.HPC, (g + 1) * cfg.HPC))
        xTc = np.ascontiguousarray(x[b].T)
        wq_c = np.concatenate(
            [W_qkv[:, gh * 3 * DK:gh * 3 * DK + DK] for gh in heads], axis=1)
        wk_c = np.concatenate(
            [W_qkv[:, gh * 3 * DK + DK:gh * 3 * DK + 2 * DK] for gh in heads], axis=1)
        wv_c = np.concatenate(
            [W_qkv[:, gh * 3 * DK + 2 * DK:gh * 3 * DK + 3 * DK] for gh in heads], axis=1)
        wo_c = np.ascontiguousarray(W_out[g * NHC:(g + 1) * NHC, :])
        bq_c = np.stack(
            [b_qkv[gh * 3 * DK:gh * 3 * DK + DK] for gh in heads], axis=1
        ) / math.sqrt(DK)
        bk_c = np.stack(
            [b_qkv[gh * 3 * DK + DK:gh * 3 * DK + 2 * DK] for gh in heads], axis=1)
        bv_flat = np.concatenate(
            [b_qkv[gh * 3 * DK + 2 * DK:gh * 3 * DK + 3 * DK] for gh in heads])
        bvb_c = np.broadcast_to(bv_flat[None, :], (P, NHC))
        in_maps.append({
            "xT": xTc.astype(np.float32),
            "wq": np.ascontiguousarray(wq_c).astype(np.float32),
            "wk": np.ascontiguousarray(wk_c).astype(np.float32),
            "wv": np.ascontiguousarray(wv_c).astype(np.float32),
            "wo": wo_c.astype(np.float32),
            "bq": np.ascontiguousarray(bq_c).astype(np.float32),
            "bk": np.ascontiguousarray(bk_c).astype(np.float32),
            "bvb": np.ascontiguousarray(bvb_c).astype(np.float32),
            "ones": np.ones((P, P), dtype=np.float32),
        })
    return in_maps


_build_lock = threading.Lock()
_cached_nc = None
LAST_RESULTS = None  # BassKernelResults of the most recent kernel() call


def _get_nc():
    global _cached_nc
    with _build_lock:
        if _cached_nc is None:
            _cached_nc = build_bass(Cfg(), num_devices=8)
    return _cached_nc


def kernel(x, W_qkv, b_qkv, W_out, b_out):
    global LAST_RESULTS
    x = np.asarray(x, dtype=np.float32)
    W_qkv = np.asarray(W_qkv, dtype=np.float32)
    b_qkv = np.asarray(b_qkv, dtype=np.float32)
    W_out = np.asarray(W_out, dtype=np.float32)
    b_out = np.asarray(b_out, dtype=np.float32)

    cfg = Cfg()
    nc = _get_nc()
    in_maps = make_in_maps(x, W_qkv, b_qkv, cfg, W_out)
    trace = bool(int(os.environ.get("KERNEL_TRACE", "0")))
    res = bass_utils.run_bass_kernel_spmd(
        nc, in_maps, core_ids=list(range(8)), trace=trace,
        stitch_traces=False,
    )
    LAST_RESULTS = res
    B = x.shape[0]
    out = np.empty((B, cfg.S, cfg.D), dtype=np.float32)
    n_groups = 8 // B
    for b in range(B):
        acc = res.results[b * n_groups]["out"].copy()
        for g in range(1, n_groups):
            acc += res.results[b * n_groups + g]["out"]
        out[b] = acc + b_out[None, :]
    return out
